# revision 33
# baseline (speedup 1.0000x reference)
"""Causal MHA (B=2, S=2048, D=2048, H=16) on 8 TRN2 NeuronCores.

Sharding: heads split across cores (2 heads/core, both batches). v2 schedule:

  1. QKV GEMM streams 512-token blocks; attention for head 0 is emitted
     interleaved after each token block (block (b,j) depends only on tokens
     <= 512*(j+1) of batch b), so softmax exp/mask/sum elementwise work rides
     under the big GEMM on Scalar/Vector/GpSimd while PE stays dense.
  2. Denominators: exp-chunk tiles are pair-summed on Vector, accumulated
     per-block on GpSimd, and reduced across partitions by ONE M=128
     ones-matmul per block -> [128,512] PSUM denominator (already broadcast
     across partitions). reciprocal+multiply normalize the block output.
     (The baseline's per-pair ones-matmuls + broadcast-matmuls serialized
     PE against the Vector engine and cost ~125us.)
  3. Causal triangle tightening: diagonal score/exp/V-matmul chunks are
     restricted to the valid sq range (saves ~20% of attention PE + exp).
  4. AllToAll is split per local head: head-0's A2A flies while head-1's
     attention computes; head-1's A2A is hidden under the first half of the
     out-projection (even k-chunks, which only need head-0's A2A).
  5. out-proj: m-chunks in 2 groups of 8 PSUM banks; even k-chunks first,
     odd k-chunks (head-1 data) after; full Wout prefetched in SBUF.

All matmul operands bf16 (f32 PSUM accumulation). Host passes x^T, per-core
W shards (attn scale folded into Wq), masks, ones, all bf16. Output is
assembled on host from the 8 transposed f32 token slices.
"""
import time

import numpy as np

import concourse.bacc as bacc
import concourse.mybir as mybir
import concourse.tile as tile
from concourse import bass_utils

# ---- problem constants (hardcoded; must match the reference) ----
B, S, D_MODEL, H = 2, 2048, 2048, 16
HEAD_DIM = 128
N_CORES = 8
CORE_IDS = list(range(N_CORES))
T = B * S                      # 4096 flattened tokens
HPC = H // N_CORES             # 2 heads per core
TOKB = 512                     # token block for phase-1 GEMM streaming
NTB = T // TOKB                # 8
NKC = D_MODEL // 128           # 16 contraction chunks of d_model
SQB = 512                      # sq block width in attention
NJ = S // SQB                  # 4 sq blocks per batch
NSK = S // 128                 # 16 sk chunks per batch
TSL = T // N_CORES             # 512-token output slice per core

F32 = mybir.dt.float32
BF16 = mybir.dt.bfloat16
EXPF = mybir.ActivationFunctionType.Exp


def build(iters: int = 1, phases: str = "1234"):
    nc = bacc.Bacc("TRN2", target_bir_lowering=False, debug=False,
                   num_devices=N_CORES)

    xT_d = nc.dram_tensor("xT", [D_MODEL, T], BF16, kind="ExternalInput").ap()
    wqk_d = nc.dram_tensor("wqk", [D_MODEL, 4 * 128], BF16, kind="ExternalInput").ap()
    wv_d = nc.dram_tensor("wv", [D_MODEL, 2 * 128], BF16, kind="ExternalInput").ap()
    wout_d = nc.dram_tensor("wout", [NKC, NKC, 128, 128], BF16, kind="ExternalInput").ap()
    mask_d = nc.dram_tensor("masks", [4, 128, SQB], BF16, kind="ExternalInput").ap()
    ones_d = nc.dram_tensor("ones", [128, 128], BF16, kind="ExternalInput").ap()
    outT_d = nc.dram_tensor("outT", [D_MODEL, TSL], F32, kind="ExternalOutput").ap()

    # internal DRAM for the per-head all-to-alls (bf16; bypass moves bytes)
    a2a_in = [nc.dram_tensor(f"a2a_in{h}", [N_CORES, 128, TSL], BF16).ap()
              for h in range(HPC)]
    a2a_out = [nc.dram_tensor(f"a2a_out{h}", [N_CORES, 128, TSL], BF16).ap()
               for h in range(HPC)]

    with tile.TileContext(nc) as tc:
        with tc.tile_pool(name="persist", bufs=1) as pp:
            masks = pp.tile([128, 4, SQB], BF16)
            ones = pp.tile([128, 128], BF16)
            wf = pp.tile([128, NKC, NKC, 128], BF16)   # full Wout, bf16
            # masks/ones + the 8MB Wout prefetch go on the gpsimd DMA queue so
            # the sync queue serves phase 1's wqk/wv/xt stream without delay;
            # wout has ~300us to land before the out-proj needs it.
            nc.gpsimd.dma_start(out=masks[:], in_=mask_d.rearrange("r p q -> p r q"))
            nc.gpsimd.dma_start(out=ones[:], in_=ones_d[:])
            wout_r = wout_d.rearrange("m k p n -> p m k n")
            for qi in range(4):
                nc.gpsimd.dma_start(out=wf[:, qi * 4:(qi + 1) * 4, :, :],
                                    in_=wout_r[:, qi * 4:(qi + 1) * 4, :, :])

            for _ in range(iters):
                _body(nc, tc, pp, xT_d, wqk_d, wv_d, outT_d,
                      a2a_in, a2a_out, masks, ones, wf, phases)

    nc.compile()
    return nc


def _body(nc, tc, pp, xT_d, wqk_d, wv_d, outT_d, a2a_in, a2a_out,
          masks, ones, wf, phases="1234"):
    # p1/p4 live on the right-side SBUF stack: their lifetimes interleave
    # with the left-side pools (p1 ends mid-attention, p4 starts there)
    qkvp = tc.alloc_tile_pool(name="qkv", bufs=1)
    psp = tc.alloc_tile_pool(name="psum", bufs=1, space="PSUM")
    p1 = tc.alloc_tile_pool(name="p1", bufs=1, side="right")
    p2 = tc.alloc_tile_pool(name="p2", bufs=1)

    # persistent activations for this iteration
    qkT = qkvp.tile([128, 4, T], BF16)      # [d, (q0,q1,k0,k1), tok]
    v_sb = qkvp.tile([128, T // 128, 2 * 128], BF16)  # [tok%128, chunk, feat]

    wqk = p1.tile([128, NKC, 4 * 128], BF16)
    wv = p1.tile([128, NKC, 2 * 128], BF16)
    # weights ride the gpsimd DMA queue: its prior per-iteration work (the
    # collectives + of-fetches) drains early in the previous out-proj, so
    # these transfers land during it and the sync queue only carries the
    # xt/epilogue stream -> next iteration's GEMM starts without a bubble
    nc.gpsimd.dma_start(out=wqk[:], in_=wqk_d.rearrange("(k p) n -> p k n", p=128))
    nc.gpsimd.dma_start(out=wv[:], in_=wv_d.rearrange("(k p) n -> p k n", p=128))
    xT_r = xT_d.rearrange("(k p) t -> p k t", p=128)

    def emit_ph1_tb(tb):
        xt = p1.tile([128, NKC, TOKB], BF16, tag="xt", bufs=2)
        # alternate sync/gpsimd rings: one ring's sustained bandwidth is
        # marginal for the 2MB/block stream at bufs=2 pacing, and the scalar
        # ring is reserved for out-proj stores (whose triggers would delay
        # the next iteration's first xt blocks)
        eng = nc.sync if tb % 2 == 0 else nc.gpsimd
        eng.dma_start(out=xt[:], in_=xT_r[:, :, tb * TOKB:(tb + 1) * TOKB])
        tok0 = tb * TOKB
        # q/k transposed GEMM: psum[feat, tok] += wqk_chunk.T @ xt_chunk
        for m in range(4):
            ps = psp.tile([128, SQB], F32, tag="mm", bufs=3)
            for kc in range(NKC):
                nc.tensor.matmul(ps[:, :TOKB],
                                 wqk[:, kc, m * 128:(m + 1) * 128],
                                 xt[:, kc, :],
                                 start=(kc == 0), stop=(kc == NKC - 1))
            # evac on DVE: ScalarE is reserved for softmax exp (its per-call
            # fixed cost makes it the attention-phase bottleneck)
            with nc.allow_low_precision(reason="bf16 qk activations"):
                nc.vector.tensor_copy(qkT[:, m, tok0:tok0 + TOKB], ps[:, :TOKB])
        # V natural GEMM: psum[tok, feat]; lhsT=xt chunk
        for ti in range(TOKB // 128):
            pv = psp.tile([128, SQB], F32, tag="pv", bufs=2)
            for kc in range(NKC):
                nc.tensor.matmul(pv[:, :256],
                                 xt[:, kc, ti * 128:(ti + 1) * 128],
                                 wv[:, kc, :],
                                 start=(kc == 0), stop=(kc == NKC - 1))
            with nc.allow_low_precision(reason="bf16 V activations"):
                nc.vector.tensor_copy(v_sb[:, tb * (TOKB // 128) + ti, :],
                                      pv[:, :256])

    # ---------------- attention (head-outer, interleaved with ph1) --------
    sts = {}
    pending = [None]

    def emit_scores(h, b, j, c):
        # diagonal chunks only need sq >= 128*(c-4j)
        off = 128 * (c - 4 * j) if c >= 4 * j else 0
        st = psp.tile([128, SQB], F32, tag="mm", bufs=3, name="st")
        nc.tensor.matmul(
            st[:, off:],
            qkT[:, 2 + h, b * S + c * 128: b * S + (c + 1) * 128],
            qkT[:, h, b * S + j * SQB + off: b * S + (j + 1) * SQB],
            start=True, stop=True)
        sts[c] = (st, off)

    def flush_epilogue(h, b, j, o_acc, bt):
        den = psp.tile([128, SQB], F32, tag="den", bufs=1, name="den")
        nc.tensor.matmul(den[:], ones[:, :], bt[:], start=True, stop=True)
        rec = p2.tile([128, SQB], BF16, tag="rec", bufs=2, name="rec")
        with nc.allow_low_precision(reason="softmax denom recip"):
            nc.vector.reciprocal(rec[:], den[:])
        o_sb = p2.tile([128, SQB], BF16, tag="osb", bufs=2, name="osb")
        with nc.allow_low_precision(reason="bf16 attn output"):
            nc.vector.tensor_mul(o_sb[:], o_acc[:], rec[:])
        nc.sync.dma_start(out=a2a_in[h][b * NJ + j, :, :], in_=o_sb[:])

    def emit_attn_block(h, b, j):
        cmax = 4 * j + 3
        o_acc = psp.tile([128, SQB], F32, tag="oacc", bufs=2, name="oacc")
        bt = p2.tile([128, SQB], BF16, tag="bt", bufs=2, name="bt")
        emit_scores(h, b, j, 0)
        if pending[0] is not None:
            flush_epilogue(*pending[0])
            pending[0] = None
        e_prev = None
        npairs = 0
        for c in range(cmax + 1):
            if c + 1 <= cmax:
                emit_scores(h, b, j, c + 1)
            st, off = sts.pop(c)
            e = p2.tile([128, SQB], BF16, tag="exp", bufs=5)
            with nc.allow_low_precision(reason="bf16 exp"):
                nc.scalar.activation(e[:, off:], st[:, off:], EXPF)
            if c >= 4 * j:
                with nc.allow_low_precision(reason="bf16 mask"):
                    nc.vector.tensor_mul(e[:, off:], e[:, off:],
                                         masks[:, c - 4 * j, off:])
            nc.tensor.matmul(
                o_acc[:, off:],
                v_sb[:, b * NSK + c, h * 128:(h + 1) * 128],
                e[:, off:], start=(c == 0), stop=(c == cmax))
            # denominator: pair-sum exps on DVE, accumulate block total on
            # GpSimd; ONE ones-matmul per block reduces across partitions
            if c % 2 == 0:
                e_prev = (e, off)
            else:
                e0, off0 = e_prev
                if off > off0:
                    nc.vector.memzero(e[:, off0:off])
                with nc.allow_low_precision(reason="bf16 den"):
                    if npairs == 0:
                        nc.vector.tensor_add(bt[:, off0:], e0[:, off0:], e[:, off0:])
                        if off0 > 0:
                            nc.vector.memzero(bt[:, :off0])
                    else:
                        pt = p2.tile([128, SQB], BF16, tag="pt", bufs=2)
                        nc.vector.tensor_add(pt[:, off0:], e0[:, off0:], e[:, off0:])
                        nc.vector.tensor_add(bt[:, off0:], bt[:, off0:], pt[:, off0:])
                npairs += 1
        pending[0] = (h, b, j, o_acc, bt)

    # ---------------- interleaved emission ----------------
    # Both heads' attention rides under the ph1 GEMM (the GEMM absorbs the
    # exp-per-chunk fixed cost on ScalarE); head-1's batch-1 blocks form the
    # tail, under which head-0's A2A flies.
    tb_blocks = {0: [(0, 0, 0)],
                 1: [(0, 0, 1), (1, 0, 0)],
                 2: [(0, 0, 2), (1, 0, 1)],
                 3: [(0, 0, 3), (1, 0, 2)],
                 4: [(0, 1, 0), (1, 0, 3)],
                 5: [(0, 1, 1)],
                 6: [(0, 1, 2), (1, 1, 0)],
                 7: [(0, 1, 3), (1, 1, 1)]}
    tail = [(1, 1, 2), (1, 1, 3)]

    for tb in range(NTB):
        emit_ph1_tb(tb)
        if "2" in phases:
            for blk in tb_blocks[tb]:
                emit_attn_block(*blk)
    p1.release()

    if "2" not in phases:
        p2.release()
        psp.release()
        qkvp.release()
        return

    # head-1 batch-1 tail; head-0 A2A flies underneath. of-DMAs ride the
    # gpsimd queue: the trigger carries the wait on the collective, and
    # would stall ScalarE's exp stream on a compute queue.
    # of/ot live in the persistent pool (not a transient right-side pool):
    # a transient pool's addresses would overlap p1's, gating the next
    # iteration's weight/xt loads on this iteration's out-proj completion
    of = pp.tile([128, 2, NKC // 2, TSL], BF16, tag="of", bufs=1)
    if "3" in phases:
        # all h0 blocks flushed by end of the tb loop ((0,1,3)'s epilogue
        # rides inside (1,1,1)'s emission) -> h0's A2A flies under the tail.
        # of-DMAs ride the gpsimd queue: the trigger carries the wait on the
        # collective, and would stall ScalarE's exp stream on its queue.
        nc.gpsimd.collective_compute(
            "AllToAll", mybir.AluOpType.bypass, replica_groups=[CORE_IDS],
            ins=[a2a_in[0][:]], outs=[a2a_out[0][:]])
        nc.gpsimd.dma_start(out=of[:, 0, :, :],
                            in_=a2a_out[0].rearrange("s p t -> p s t"))
    for (h, b, j) in tail:
        emit_attn_block(h, b, j)
    flush_epilogue(*pending[0])
    pending[0] = None
    if "3" in phases:
        nc.gpsimd.collective_compute(
            "AllToAll", mybir.AluOpType.bypass, replica_groups=[CORE_IDS],
            ins=[a2a_in[1][:]], outs=[a2a_out[1][:]])
        nc.gpsimd.dma_start(out=of[:, 1, :, :],
                            in_=a2a_out[1].rearrange("s p t -> p s t"))
    else:
        for h in range(HPC):
            nc.gpsimd.dma_start(out=of[:, h, :, :],
                                in_=a2a_in[h].rearrange("s p t -> p s t"))

    p2.release()
    psp.release()
    qkvp.release()

    # ---------------- out-projection ----------------
    if "4" not in phases:
        return
    psp4 = tc.alloc_tile_pool(name="psum4", bufs=1, space="PSUM")
    for g in range(2):
        pos = []
        # even k-chunks (head-0 data) for all 8 m's of this group first:
        # runs while head-1's A2A is still in flight
        for m in range(g * 8, (g + 1) * 8):
            po = psp4.tile([128, TSL], F32, tag="po", bufs=8)
            pos.append(po)
            for kc in range(0, NKC, 2):
                nc.tensor.matmul(po[:], wf[:, m, kc, :], of[:, 0, kc // 2, :],
                                 start=(kc == 0), stop=False)
        for m in range(g * 8, (g + 1) * 8):
            po = pos[m - g * 8]
            for kc in range(1, NKC, 2):
                nc.tensor.matmul(po[:], wf[:, m, kc, :], of[:, 1, kc // 2, :],
                                 start=False, stop=(kc == NKC - 1))
            ot = pp.tile([128, TSL], F32, tag="ot", bufs=3)
            nc.scalar.copy(ot[:], po[:])
            # triple-buffered ot decouples the evac chain from the ~2.8us
            # per-256KB store; all stores on the scalar ring so the sync
            # ring only ever carries the xt stream + epilogues
            nc.scalar.dma_start(out=outT_d[m * 128:(m + 1) * 128, :], in_=ot[:])
    psp4.release()


def _host_inputs(x, Wqkv, Wout):
    bf = mybir.dt.np(BF16)
    xT = np.ascontiguousarray(x.reshape(T, D_MODEL).T).astype(bf)
    scale = np.float32(HEAD_DIM ** -0.5)
    masks1 = np.zeros((4, 128, SQB), dtype=np.float32)
    for r in range(4):
        for i in range(128):
            lo = i + 128 * r
            if lo < SQB:
                masks1[r, i, lo:] = 1.0
    masks = masks1.astype(bf)
    ones = np.ones((128, 128), dtype=bf)
    # [m, k, 128, 128]: tile (k,m) of Wout, so each m-chunk load is contiguous
    Wout_t = np.ascontiguousarray(
        Wout.astype(np.float32).reshape(NKC, 128, NKC, 128).transpose(2, 0, 1, 3)
    ).astype(bf)

    in_maps = []
    for c in range(N_CORES):
        cols_q = [Wqkv[:, (2 * c + h) * 128:(2 * c + h + 1) * 128] for h in range(HPC)]
        cols_k = [Wqkv[:, D_MODEL + (2 * c + h) * 128:D_MODEL + (2 * c + h + 1) * 128]
                  for h in range(HPC)]
        cols_v = [Wqkv[:, 2 * D_MODEL + (2 * c + h) * 128:2 * D_MODEL + (2 * c + h + 1) * 128]
                  for h in range(HPC)]
        wqk = np.concatenate([c_ * scale for c_ in cols_q] + cols_k, axis=1)
        wv = np.concatenate(cols_v, axis=1)
        in_maps.append({
            "xT": xT,
            "wqk": np.ascontiguousarray(wqk.astype(np.float32)).astype(bf),
            "wv": np.ascontiguousarray(wv.astype(np.float32)).astype(bf),
            "wout": Wout_t,
            "masks": masks,
            "ones": ones,
        })
    return in_maps


_NC_CACHE = {}


def _get_nc(iters=1, phases="1234"):
    key = (iters, phases)
    if key not in _NC_CACHE:
        _NC_CACHE[key] = build(iters, phases)
    return _NC_CACHE[key]


def kernel(x, Wqkv, Wout):
    x = np.asarray(x, dtype=np.float32)
    Wqkv = np.asarray(Wqkv, dtype=np.float32)
    Wout = np.asarray(Wout, dtype=np.float32)
    nc = _get_nc(1)
    in_maps = _host_inputs(x, Wqkv, Wout)
    res = None
    for attempt in range(3):
        try:
            res = bass_utils.run_bass_kernel_spmd(nc, in_maps, CORE_IDS)
            break
        except Exception:
            # transient NRT_EXEC_UNIT_UNRECOVERABLE after heavy prior device
            # activity recovers on retry; re-raise if persistent
            if attempt == 2:
                raise
            time.sleep(20)
    outT = np.concatenate([res.results[c]["outT"] for c in range(N_CORES)], axis=1)
    return np.ascontiguousarray(outT.T).reshape(B, S, D_MODEL)
